# revision 10
# baseline (speedup 1.0000x reference)
"""Masked graph-attention aggregator on 8 Trainium2 NeuronCores (Bass/Tile).

Computation:
    q/k/v = x @ W{q,k,v}.T + b                     -> [H=8, N=4096, DH=32]
    att   = softmax(mask(q k^T / sqrt(DH)))        mask from edge_index
    y     = att @ v                                -> [N, 256]
    out   = concat([x, y], 1) @ Wp.T + bp          -> [N, 256]

Sharding: query rows split 512 per core; x^T / W^T and a dense bf16 0/1
mask^T [4096 keys, 512 queries] are prepared host-side (pure input
relayout).  The V bias is folded into the output-projection bias.

Per core the masked-softmax work (8 heads x 512 q x 4096 k scores) is
split across three engines so none is the lone bottleneck:
  - A-route (ACT): exp straight out of PSUM (free affine = 1/sqrt(DH)),
    mask-multiply on DVE (stride-0 broadcast AP reuses one mask for both
    heads) or on GPSIMD (plain 2D multiplies).  Score tiles come BIG
    [128,2048] (2 kc x 2 heads, amortizing ACTIVATE overhead) or SMALL
    [128,1024], ping-ponged across two PSUM regions.
  - B-route (no ACT): exp(s) ~ (1+s/2)^2 = 1 + 2U + U^2 with
    U = (S*a)(.)M via two fused scalar_tensor_tensor passes on DVE;
    the leftover mask term rides the PV matmuls (V^T M, head-shared
    rhs).  End-to-end rel err of the approximation ~8e-4.
  - PV: per (group, kc) two matmuls with [V_h | ones] 33-column blocks:
    one instruction yields both the 32 y rows and the softmax
    denominator row at a 32-aligned offset (h0 -> rows 0-32 at col
    strip 0, h1 -> rows 64-96 at strip 2).  pv is a single PSUM bank.
    The scrambled y-row layout is undone for free by zero-padded
    per-group Wp row blocks in the final projection.
  - Drains: reciprocal_approx_fast directly on the PSUM Z rows, one
    sel-matmul per group broadcasts 1/Z to the y rows (M=128 so unused
    rp rows are zeroed); warm-up matmuls pre-fill pv so untouched
    partitions stay finite.  Biases ride rank-1 matmuls on the PE.
"""

import numpy as np
import ml_dtypes

import concourse.bass as bass
import concourse.mybir as mybir
import concourse.tile as tile
from concourse import library_config
from concourse.library_overlay import lower_extended_insts

N = 4096
D = 256
H = 8
DH = 32
NCORES = 8
QR = N // NCORES  # 512
NKC = N // 128    # 32 key chunks
VB = 34           # Vaug block stride per head (32 V cols + ones + pad)
SCALE = 1.0 / float(np.sqrt(np.float32(DH)))
A2 = SCALE / 2.0  # taylor coefficient: exp(s) ~ (1 + a*S)^2, S = raw qk

f32 = mybir.dt.float32
bf16 = mybir.dt.bfloat16

AF = mybir.ActivationFunctionType
OP = mybir.AluOpType

# per-group small-unit type schedule (12 smalls at kc 2,5,8,...,29,30,31)
# 'S': exp on ACT + mask on DVE;  'G': exp on ACT + mask on GPSIMD;
# 'B': taylor fully on DVE (2 fused passes), mask-term via PE.
SMALLS_G0 = ['G', 'G', 'G', 'G', 'B', 'G', 'G', 'G', 'G', 'B', 'G', 'G']
SMALLS_GX = ['B', 'G', 'B', 'G', 'B', 'G', 'B', 'G', 'B', 'G', 'B', 'G']
# big units (index 0..9 covering kc 3i,3i+1) whose mask runs on GPSIMD
BIGG_IDX = {4, 7}


def _unit_list(g):
    """Units covering kc 0..31: (kind, kc) with BIG kinds covering kc,kc+1."""
    smalls = SMALLS_G0 if g == 0 else SMALLS_GX
    units = []
    kc = 0
    si = 0
    for bi in range(10):         # (BIG, small) x 10 -> 3 kc each
        units.append(('BIGG' if bi in BIGG_IDX else 'BIG', kc)); kc += 2
        units.append((smalls[si], kc)); kc += 1
        si += 1
    units.append((smalls[si], kc)); kc += 1; si += 1
    units.append((smalls[si], kc)); kc += 1; si += 1
    assert kc == NKC and si == 12
    return units


def _split_multi_waits(nc):
    """Walrus encodes at most one sync-wait per instruction; move extras onto
    single-wait NoOps inserted before the instruction on the same engine."""
    ctr = 0
    for f in nc.m.functions:
        for bb in f.blocks:
            il = bb.instructions
            i = 0
            while i < len(il):
                ins = il[i]
                si = ins.sync_info
                if si is not None and len(si.on_wait) > 1:
                    waits = list(si.on_wait)
                    ins.sync_info = mybir.SyncInfo(
                        on_wait=[waits[-1]], on_update=list(si.on_update)
                    )
                    for w in waits[:-1]:
                        ctr += 1
                        nop = mybir.InstNoOp(
                            name=f"I-waitsplit-{ctr}", ins=[], outs=[]
                        )
                        nop.engine = ins.engine
                        nop.sync_info = mybir.SyncInfo(on_wait=[w], on_update=[])
                        il.insert(i, nop)
                        i += 1
                i += 1


def build_program(split: bool = True, debug: bool = False) -> bass.Bass:
    nc = bass.Bass()

    xT_in = nc.dram_tensor("xT_in", [D, N], bf16, kind="ExternalInput")
    xrT_in = nc.dram_tensor("xrT_in", [D, QR], bf16, kind="ExternalInput")
    wqT_in = nc.dram_tensor("wqT_in", [D, D], bf16, kind="ExternalInput")
    wkT_in = nc.dram_tensor("wkT_in", [D, D], bf16, kind="ExternalInput")
    wvT_in = nc.dram_tensor("wvT_in", [D, D], bf16, kind="ExternalInput")
    # [x rows (256) | per-group y row blocks (4 x 128, zero-padded)]
    wpT_in = nc.dram_tensor("wpT_in", [6 * 128, D], bf16, kind="ExternalInput")
    bq_in = nc.dram_tensor("bq_in", [1, D], bf16, kind="ExternalInput")
    bk_in = nc.dram_tensor("bk_in", [1, D], bf16, kind="ExternalInput")
    bp_in = nc.dram_tensor("bp_in", [1, D], bf16, kind="ExternalInput")
    mask_in = nc.dram_tensor("mask_in", [128, NKC * QR], bf16, kind="ExternalInput")
    out = nc.dram_tensor("out", [QR, D], f32, kind="ExternalOutput")
    if debug:
        dbg = {
            nm: nc.dram_tensor(f"dbg_{nm}", shp, dt, kind="ExternalOutput")
            for nm, shp, dt in [
                ("qt0", [128, QR], bf16),
                ("kt0", [128, QR], bf16),
                ("vaug", [128, 2 * 272], bf16),
                ("mask", [128, 1024], bf16),
                ("praw", [128, 2048], bf16),
                ("phat", [128, 2048], bf16),
                ("u", [128, 1024], bf16),
                ("p2", [128, 1024], bf16),
                ("yst0", [128, QR], f32),
                ("rzb0", [66, QR], bf16),
                ("yg0", [128, QR], bf16),
            ]
        }

    with tile.TileContext(nc) as tc:
        with (
            tc.tile_pool(name="cons", bufs=1) as cons,
            tc.tile_pool(name="big", bufs=1) as big,
            tc.tile_pool(name="work", bufs=3) as work,
        ):
            nc.gpsimd.load_library(library_config.standard)

            xrT = [cons.tile([128, QR], bf16, tag=f"xrT{i}", name=f"xrT{i}") for i in range(2)]
            wqT = [cons.tile([128, D], bf16, tag=f"wqT{i}", name=f"wqT{i}") for i in range(2)]
            xT = [cons.tile([128, N], bf16, tag=f"xT{i}", name=f"xT{i}") for i in range(2)]
            wkT = [cons.tile([128, D], bf16, tag=f"wkT{i}", name=f"wkT{i}") for i in range(2)]
            wvT = [cons.tile([128, D], bf16, tag=f"wvT{i}", name=f"wvT{i}") for i in range(2)]
            wpT = [cons.tile([128, D], bf16, tag=f"wpT{i}", name=f"wpT{i}") for i in range(6)]
            for i in range(2):
                nc.sync.dma_start(out=xrT[i][:], in_=xrT_in[i * 128 : (i + 1) * 128, :])
                nc.sync.dma_start(out=wqT[i][:], in_=wqT_in[i * 128 : (i + 1) * 128, :])
            for i in range(2):
                nc.sync.dma_start(out=wkT[i][:], in_=wkT_in[i * 128 : (i + 1) * 128, :])
                nc.sync.dma_start(out=xT[i][:], in_=xT_in[i * 128 : (i + 1) * 128, :])
            for i in range(2):
                nc.sync.dma_start(out=wvT[i][:], in_=wvT_in[i * 128 : (i + 1) * 128, :])
            # dense 0/1 mask^T, kc-chunk DMAs right behind the operands that
            # gate the PE start so chunk k lands before unit k consumes it
            mask_sb = big.tile([128, NKC * QR], bf16)
            for kc in range(NKC):
                nc.sync.dma_start(
                    out=mask_sb[:, kc * QR : (kc + 1) * QR],
                    in_=mask_in[:, kc * QR : (kc + 1) * QR],
                )
            for i in range(6):
                nc.sync.dma_start(out=wpT[i][:], in_=wpT_in[i * 128 : (i + 1) * 128, :])
            bq_sb = cons.tile([1, D], bf16)
            bk_sb = cons.tile([1, D], bf16)
            bp_sb = cons.tile([1, D], bf16)
            nc.sync.dma_start(out=bq_sb[:], in_=bq_in[:])
            nc.sync.dma_start(out=bk_sb[:], in_=bk_in[:])
            nc.sync.dma_start(out=bp_sb[:], in_=bp_in[:])

            ones_bf = cons.tile([1, QR], bf16)
            nc.vector.memset(ones_bf[:], 1.0)
            ones128 = cons.tile([128, 128], bf16)
            nc.vector.memset(ones128[:], 1.0)
            # sel: 1/Z broadcast.  Z rows 32 (head0) / 96 (head1) fan out to
            # y rows 0-31 / 64-95; all other output rows get zeros.
            sel_sb = cons.tile([128, 128], bf16)
            nc.vector.memset(sel_sb[:], 0.0)
            nc.vector.memset(sel_sb[32:33, 0:32], 1.0)
            nc.vector.memset(sel_sb[96:97, 64:96], 1.0)

            QT = [big.tile([128, QR], bf16, tag=f"QT{i}", name=f"QT{i}") for i in range(2)]
            KT = [big.tile([128, N], bf16, tag=f"KT{i}", name=f"KT{i}") for i in range(2)]
            # V blocks: [128 nodes, kc*8*VB + h*VB + (32 V cols | ones | pad)]
            Vaug = big.tile([128, NKC * H * VB], bf16)
            vv = Vaug[:].rearrange("p (kc h c) -> p kc h c", kc=NKC, h=H)
            nc.vector.memset(vv[:, :, :, 32:33], 1.0)

            ystage = [
                cons.tile([128, QR], f32, tag=f"yst{i}", name=f"yst{i}")
                for i in range(4)
            ]
            yg = [cons.tile([128, QR], bf16, tag=f"yg{i}", name=f"yg{i}") for i in range(4)]
            rz = cons.tile([128, QR], f32, tag="rz", name="rz")
            rzb = cons.tile([128, QR], bf16, tag="rzb", name="rzb")

            def mask3(kc):
                return (
                    mask_sb[:, kc * QR : (kc + 1) * QR]
                    .unsqueeze(1)
                    .broadcast_to([128, 2, QR])
                )

            def mask4(kc0):
                return (
                    mask_sb[:, kc0 * QR : (kc0 + 2) * QR]
                    .rearrange("p (kc q) -> p kc q", kc=2)
                    .unsqueeze(2)
                    .broadcast_to([128, 2, 2, QR])
                )

            with tc.tile_pool(name="psa", bufs=1, space="PSUM") as psa:
                pvt = [None]

                def st_mm(sp_slice, g, kc, h):
                    band = (h % 4) * DH
                    dt_ = g // 2
                    nc.tensor.matmul(
                        sp_slice,
                        lhsT=KT[dt_][band : band + DH, kc * 128 : (kc + 1) * 128],
                        rhs=QT[dt_][band : band + DH, :],
                        start=True,
                        stop=True,
                        tile_position=(band, 0),
                    )

                def pv_mm(g, kc, rhs_pair, first, last, rhs_is_mask=False):
                    """Two PV accumulants: head j of group g contributes its
                    32 y rows + Z row via a 33-col [V|ones] block."""
                    pv = pvt[0]
                    for j in range(2):
                        h = 2 * g + j
                        base = (kc * H + h) * VB
                        rhs = rhs_pair if rhs_is_mask else rhs_pair[:, j * QR : (j + 1) * QR]
                        nc.tensor.matmul(
                            pv[64 * j : 64 * j + 33, :],
                            lhsT=Vaug[:, base : base + 33],
                            rhs=rhs,
                            start=first,
                            stop=last,
                            tile_position=(0, 64 * j),
                        )

                def emit_big(g, kc0, last, on_gp=False):
                    h0, h1 = 2 * g, 2 * g + 1
                    sp = psa.tile([128, 2048], f32, tag="spb", bufs=1)
                    for j, (kc, h) in enumerate(
                        ((kc0, h0), (kc0, h1), (kc0 + 1, h0), (kc0 + 1, h1))
                    ):
                        st_mm(sp[:, j * QR : (j + 1) * QR], g, kc, h)
                    praw = work.tile([128, 2048], bf16, tag="prb", bufs=2)
                    nc.scalar.activation(praw[:], sp[:], AF.Exp, scale=SCALE)
                    phat = work.tile([128, 2048], bf16, tag="phb", bufs=2)
                    if on_gp:
                        for j, kc in enumerate((kc0, kc0, kc0 + 1, kc0 + 1)):
                            nc.gpsimd.tensor_tensor(
                                out=phat[:, j * QR : (j + 1) * QR],
                                in0=praw[:, j * QR : (j + 1) * QR],
                                in1=mask_sb[:, kc * QR : (kc + 1) * QR],
                                op=OP.mult,
                            )
                    else:
                        nc.vector.tensor_tensor(
                            out=phat[:].rearrange("p (a b q) -> p a b q", a=2, b=2),
                            in0=praw[:].rearrange("p (a b q) -> p a b q", a=2, b=2),
                            in1=mask4(kc0),
                            op=OP.mult,
                        )
                    if debug and g == 0 and kc0 == 0:
                        nc.sync.dma_start(out=dbg["praw"][:], in_=praw[:])
                        nc.sync.dma_start(out=dbg["phat"][:], in_=phat[:])
                    # big layout: [kc0 h0 | kc0 h1 | kc1 h0 | kc1 h1]; pv
                    # wants per-kc [h0 | h1] pairs
                    pv_mm(g, kc0, phat[:, 0:1024], kc0 == 0, False)
                    pv_mm(g, kc0 + 1, phat[:, 1024:2048], False, last)

                def emit_small(g, kc, kind, last):
                    h0, h1 = 2 * g, 2 * g + 1
                    sp = psa.tile([128, 1024], f32, tag="sps", bufs=1)
                    st_mm(sp[:, 0:QR], g, kc, h0)
                    st_mm(sp[:, QR : 2 * QR], g, kc, h1)
                    if kind == 'B':
                        U = work.tile([128, 1024], bf16, tag="ub", bufs=2)
                        nc.vector.scalar_tensor_tensor(
                            out=U[:].rearrange("p (a q) -> p a q", a=2),
                            in0=sp[:].rearrange("p (a q) -> p a q", a=2),
                            scalar=A2,
                            in1=mask3(kc),
                            op0=OP.mult,
                            op1=OP.mult,
                        )
                        P2 = work.tile([128, 1024], bf16, tag="p2b", bufs=2)
                        nc.vector.scalar_tensor_tensor(
                            out=P2[:],
                            in0=U[:],
                            scalar=2.0,
                            in1=U[:],
                            op0=OP.add,
                            op1=OP.mult,
                        )
                        if debug and g == 0 and kc == 14:
                            nc.sync.dma_start(out=dbg["u"][:], in_=U[:])
                            nc.sync.dma_start(out=dbg["p2"][:], in_=P2[:])
                        pv_mm(g, kc, P2[:], kc == 0, False)
                        # mask term: exp(s) ~ 1 + 2U + U^2, the "1" rides the
                        # PE with the head-shared mask as rhs
                        pv_mm(
                            g, kc, mask_sb[:, kc * QR : (kc + 1) * QR],
                            False, last, rhs_is_mask=True,
                        )
                    else:
                        praw = work.tile([128, 1024], bf16, tag="prs", bufs=2)
                        nc.scalar.activation(praw[:], sp[:], AF.Exp, scale=SCALE)
                        phat = work.tile([128, 1024], bf16, tag="phs", bufs=2)
                        if kind == 'G':
                            mkc = mask_sb[:, kc * QR : (kc + 1) * QR]
                            for c0 in (0, QR):
                                nc.gpsimd.tensor_tensor(
                                    out=phat[:, c0 : c0 + QR],
                                    in0=praw[:, c0 : c0 + QR],
                                    in1=mkc,
                                    op=OP.mult,
                                )
                        else:
                            nc.vector.tensor_tensor(
                                out=phat[:].rearrange("p (a q) -> p a q", a=2),
                                in0=praw[:].rearrange("p (a q) -> p a q", a=2),
                                in1=mask3(kc),
                                op=OP.mult,
                            )
                        pv_mm(g, kc, phat[:], kc == 0, last)

                def emit_group(g):
                    for kind, kc in _unit_list(g):
                        if kind in ('BIG', 'BIGG'):
                            emit_big(g, kc, kc + 1 == NKC - 1, kind == 'BIGG')
                        else:
                            emit_small(g, kc, kind, kc == NKC - 1)

                def drain_group(g, rp):
                    pv = pvt[0]
                    nc.vector.tensor_copy(ystage[g][:], pv[:])
                    # K=98 contraction from partition 0 (row-offset
                    # tile_position requires K<=64); rows with zero sel
                    # weights contribute nothing, and recip of the finite
                    # non-Z rows stays finite.
                    nc.vector.reciprocal_approx_fast(rz[0:98, :], pv[0:98, :])
                    nc.vector.tensor_copy(rzb[0:98, :], rz[0:98, :])
                    nc.tensor.matmul(
                        rp[:],
                        lhsT=sel_sb[0:98, :],
                        rhs=rzb[0:98, :],
                        start=True,
                        stop=True,
                        tile_position=(0, 0),
                    )
                    if debug and g == 0:
                        nc.sync.dma_start(out=dbg["yst0"][:], in_=ystage[0][:])
                        nc.sync.dma_start(out=dbg["rzb0"][:], in_=rzb[32:98, :])
                    nc.vector.tensor_tensor(
                        out=yg[g][:], in0=ystage[g][:], in1=rp[:], op=OP.mult
                    )
                    if debug and g == 0:
                        nc.sync.dma_start(out=dbg["yg0"][:], in_=yg[0][:])

                with tc.tile_pool(name="psp", bufs=1, space="PSUM") as psp:
                    # PE warm-up doubles as pv pre-fill: every partition of pv
                    # gets a finite value before the Z-recip reads untouched
                    # rows.
                    pvt[0] = psa.tile([128, QR], f32, tag="pv", bufs=1, name="pv_g0")
                    for r in range(32):
                        j = r % 4
                        nc.tensor.matmul(
                            pvt[0][:, j * 128 : (j + 1) * 128],
                            lhsT=ones128[:],
                            rhs=ones128[:],
                            start=True,
                            stop=True,
                        )

                    def bias_mm(ps, brow, dt_):
                        nc.tensor.matmul(
                            ps,
                            lhsT=brow[0:1, dt_ * 128 : (dt_ + 1) * 128],
                            rhs=ones_bf[0:1, :],
                            start=False,
                            stop=True,
                            tile_position=(0, 0),
                        )

                    # Q projection
                    for dt_ in range(2):
                        qp = psp.tile([128, QR], f32, tag="proj", bufs=1)
                        for cc in range(2):
                            nc.tensor.matmul(
                                qp[:],
                                lhsT=wqT[cc][:, dt_ * 128 : (dt_ + 1) * 128],
                                rhs=xrT[cc][:],
                                start=(cc == 0),
                                stop=False,
                            )
                        bias_mm(qp[:], bq_sb, dt_)
                        nc.vector.tensor_copy(QT[dt_][:], qp[:])
                        if debug and dt_ == 0:
                            nc.sync.dma_start(out=dbg["qt0"][:], in_=QT[0][:])

                    # K/V projections interleaved with group-0 attention
                    units0 = _unit_list(0)
                    ui = 0
                    for nch in range(8):
                        for dt_ in range(2):
                            kp = psp.tile([128, QR], f32, tag="proj", bufs=1)
                            for cc in range(2):
                                nc.tensor.matmul(
                                    kp[:],
                                    lhsT=wkT[cc][:, dt_ * 128 : (dt_ + 1) * 128],
                                    rhs=xT[cc][:, nch * QR : (nch + 1) * QR],
                                    start=(cc == 0),
                                    stop=False,
                                )
                            bias_mm(kp[:], bk_sb, dt_)
                            nc.vector.tensor_copy(
                                KT[dt_][:, nch * QR : (nch + 1) * QR], kp[:]
                            )
                            if debug and nch == 0 and dt_ == 0:
                                nc.sync.dma_start(
                                    out=dbg["kt0"][:], in_=KT[0][:, 0:QR]
                                )
                        for nb in range(nch * 4, nch * 4 + 4):
                            vp = psp.tile([128, QR], f32, tag="proj", bufs=1)
                            for cc in range(2):
                                nc.tensor.matmul(
                                    vp[:, 0:D],
                                    lhsT=xT[cc][:, nb * 128 : (nb + 1) * 128],
                                    rhs=wvT[cc][:],
                                    start=(cc == 0),
                                    stop=(cc == 1),
                                )
                            nc.vector.tensor_copy(
                                Vaug[:, nb * H * VB : (nb + 1) * H * VB]
                                .rearrange("p (h c) -> p h c", h=H)[:, :, 0:32],
                                vp[:, 0:D].rearrange("p (h c) -> p h c", h=H),
                            )
                            if debug and nb == 1:
                                nc.sync.dma_start(
                                    out=dbg["vaug"][:], in_=Vaug[:, 0 : 2 * H * VB]
                                )
                                nc.sync.dma_start(
                                    out=dbg["mask"][:], in_=mask_sb[:, 0:1024]
                                )
                        kc_ready = nch * 4 + 4
                        while ui < len(units0):
                            kind, kc = units0[ui]
                            span = 2 if kind in ('BIG', 'BIGG') else 1
                            if kc + span > kc_ready:
                                break
                            if kind in ('BIG', 'BIGG'):
                                emit_big(0, kc, kc + 1 == NKC - 1, kind == 'BIGG')
                            else:
                                emit_small(0, kc, kind, kc == NKC - 1)
                            ui += 1
                    assert ui == len(units0)

                with tc.tile_pool(name="psr", bufs=1, space="PSUM") as psr:
                    rp = psr.tile([128, QR], f32, tag="rp", bufs=1, name="rp")
                    drain_group(0, rp)
                    for g in range(1, 4):
                        pvt[0] = psa.tile(
                            [128, QR], f32, tag="pv", bufs=1, name=f"pv_g{g}"
                        )
                        emit_group(g)
                        drain_group(g, rp)

            # ---------- final projection ----------
            with tc.tile_pool(name="pso", bufs=1, space="PSUM") as pso:
                catT = [xrT[0], xrT[1], yg[0], yg[1], yg[2], yg[3]]
                for qb in range(4):
                    op_ = pso.tile([128, D], f32, tag="op", bufs=2)
                    for cc in range(6):
                        nc.tensor.matmul(
                            op_[:],
                            lhsT=catT[cc][:, qb * 128 : (qb + 1) * 128],
                            rhs=wpT[cc][:],
                            start=(cc == 0),
                            stop=False,
                        )
                    nc.tensor.matmul(
                        op_[:],
                        lhsT=ones_bf[:, 0:128],
                        rhs=bp_sb[:],
                        start=False,
                        stop=True,
                    )
                    osb = work.tile([128, D], f32, tag="osb")
                    nc.vector.tensor_copy(osb[:], op_[:])
                    nc.sync.dma_start(
                        out=out[qb * 128 : (qb + 1) * 128, :], in_=osb[:]
                    )

    lower_extended_insts(nc)
    if split:
        _split_multi_waits(nc)
    return nc


_PROGRAM = None


def _get_program():
    global _PROGRAM
    if _PROGRAM is None:
        _PROGRAM = build_program()
    return _PROGRAM


def shard_inputs(inputs):
    bf = ml_dtypes.bfloat16
    x = np.asarray(inputs["x"], np.float32)
    ei = np.asarray(inputs["edge_index"])
    src = ei[0].astype(np.int64)   # query node of each edge
    dst = ei[1].astype(np.int64)   # key node of each edge
    Wq = np.asarray(inputs["Wq"], np.float32)
    Wk = np.asarray(inputs["Wk"], np.float32)
    Wv = np.asarray(inputs["Wv"], np.float32)
    Wp = np.asarray(inputs["Wp"], np.float32)
    bq = np.asarray(inputs["bq"], np.float32)
    bk = np.asarray(inputs["bk"], np.float32)
    bv = np.asarray(inputs["bv"], np.float32)
    bp = np.asarray(inputs["bp"], np.float32)

    xT = np.ascontiguousarray(x.T.astype(bf))                      # [256, 4096]
    wqT = np.ascontiguousarray(Wq.T.astype(bf))                    # [256, 256]
    wkT = np.ascontiguousarray(Wk.T.astype(bf))
    wvT = np.ascontiguousarray(Wv.T.astype(bf))
    bq1 = np.ascontiguousarray(bq.reshape(1, D).astype(bf))
    bk1 = np.ascontiguousarray(bk.reshape(1, D).astype(bf))
    # softmax rows sum to 1: fold the V bias into the output bias.
    bp_f = bp + bv @ Wp[:, D : 2 * D].T
    bp1 = np.ascontiguousarray(bp_f.reshape(1, D).astype(bf))

    # wpT blocks: x rows then per-group y rows at their pv positions
    # (yg_g row r = head 2g dim r; row 64+r = head 2g+1 dim r; rest zero)
    WpT = Wp.T                                                      # [512, 256]
    wp6 = np.zeros((6 * 128, D), np.float32)
    wp6[0:256] = WpT[0:256]
    for g in range(4):
        blk = wp6[(2 + g) * 128 : (3 + g) * 128]
        blk[0:32] = WpT[D + 64 * g : D + 64 * g + 32]
        blk[64:96] = WpT[D + 64 * g + 32 : D + 64 * g + 64]
    wp6 = np.ascontiguousarray(wp6.astype(bf))

    # dense 0/1 mask^T: mask[k_local, kc*QR + q] = edge(query q0+q -> key)
    allowed = np.zeros((N, N), np.bool_)                           # [key, query]
    allowed[dst, src] = True

    in_maps = []
    for c in range(NCORES):
        q0 = c * QR
        mc = allowed[:, q0 : q0 + QR]                              # [4096, 512]
        mh = np.ascontiguousarray(
            mc.reshape(NKC, 128, QR).transpose(1, 0, 2).reshape(128, NKC * QR)
            .astype(bf)
        )
        in_maps.append(
            {
                "xT_in": xT,
                "xrT_in": np.ascontiguousarray(xT[:, q0 : q0 + QR]),
                "wqT_in": wqT, "wkT_in": wkT, "wvT_in": wvT, "wpT_in": wp6,
                "bq_in": bq1, "bk_in": bk1, "bp_in": bp1,
                "mask_in": mh,
            }
        )
    return in_maps


def run(inputs, trace=False):
    from concourse.bass_utils import run_bass_kernel_spmd

    nc = _get_program()
    in_maps = shard_inputs(inputs)
    res = run_bass_kernel_spmd(nc, in_maps, core_ids=list(range(NCORES)), trace=trace)
    full = np.concatenate([res.results[c]["out"] for c in range(NCORES)], axis=0)
    return np.ascontiguousarray(full.astype(np.float32)), res


def kernel(**inputs) -> np.ndarray:
    out, _ = run(inputs, trace=False)
    return out


# revision 14
# speedup vs baseline: 1.0510x; 1.0510x over previous
"""Masked graph-attention aggregator on 8 Trainium2 NeuronCores (Bass/Tile).

Computation:
    q/k/v = x @ W{q,k,v}.T + b                     -> [H=8, N=4096, DH=32]
    att   = softmax(mask(q k^T / sqrt(DH)))        mask from edge_index
    y     = att @ v                                -> [N, 256]
    out   = concat([x, y], 1) @ Wp.T + bp          -> [N, 256]

Sharding: query rows split 512 per core; x^T / W^T and a dense bf16 0/1
mask^T [4096 keys, 512 queries] are prepared host-side (pure input
relayout).  The V bias is folded into the output-projection bias.

Per core the masked-softmax work (8 heads x 512 q x 4096 k scores) is
split across three engines so none is the lone bottleneck:
  - A-route (ACT): exp straight out of PSUM (free affine = 1/sqrt(DH)),
    mask-multiply on DVE (stride-0 broadcast AP reuses one mask for both
    heads) or on GPSIMD (plain 2D multiplies).  Score tiles come BIG
    [128,2048] (2 kc x 2 heads, amortizing ACTIVATE overhead) or SMALL
    [128,1024], ping-ponged across two PSUM regions.
  - B-route (no ACT): exp(s) ~ (1+s/2)^2 = 1 + 2U + U^2 with
    U = (S*a)(.)M via two fused scalar_tensor_tensor passes on DVE;
    the leftover mask term rides the PV matmuls (V^T M, head-shared
    rhs).  End-to-end rel err of the approximation ~8e-4.
  - PV: per (group, kc) two matmuls with [V_h | ones] 33-column blocks:
    one instruction yields both the 32 y rows and the softmax
    denominator row at a 32-aligned offset (h0 -> rows 0-32 at col
    strip 0, h1 -> rows 64-96 at strip 2).  pv is a single PSUM bank.
    The scrambled y-row layout is undone for free by zero-padded
    per-group Wp row blocks in the final projection.
  - Drains: reciprocal_approx_fast directly on the PSUM Z rows, one
    sel-matmul per group broadcasts 1/Z to the y rows (M=128 so unused
    rp rows are zeroed); warm-up matmuls pre-fill pv so untouched
    partitions stay finite.  Biases ride rank-1 matmuls on the PE.
"""

import numpy as np
import ml_dtypes

import concourse.bass as bass
import concourse.mybir as mybir
import concourse.tile as tile
from concourse import library_config
from concourse.library_overlay import lower_extended_insts

N = 4096
D = 256
H = 8
DH = 32
NCORES = 8
QR = N // NCORES  # 512
NKC = N // 128    # 32 key chunks
VB = 34           # Vaug block stride per head (32 V cols + ones + pad)
SCALE = 1.0 / float(np.sqrt(np.float32(DH)))
A2 = SCALE / 2.0  # taylor coefficient: exp(s) ~ (1 + a*S)^2, S = raw qk

f32 = mybir.dt.float32
bf16 = mybir.dt.bfloat16
f8 = mybir.dt.float8e4

AF = mybir.ActivationFunctionType
OP = mybir.AluOpType

# per-group small-unit type schedule (12 smalls at kc 2,5,8,...,29,30,31)
# 'S': exp on ACT + mask on DVE;  'G': exp on ACT + mask on GPSIMD;
# 'B': taylor fully on DVE (2 fused passes), mask-term via PE.
SMALLS_G0 = ['G', 'G', 'G', 'G', 'B', 'G', 'G', 'G', 'G', 'B', 'G', 'G']
SMALLS_GX = ['B', 'G', 'B', 'G', 'B', 'G', 'B', 'G', 'G', 'G', 'G', 'G']
# big units (index 0..9 covering kc 3i,3i+1) whose mask runs on GPSIMD
BIGG_IDX = {4, 7}


def _unit_list(g):
    """Units covering kc 0..31: (kind, kc) with BIG kinds covering kc,kc+1."""
    smalls = SMALLS_G0 if g == 0 else SMALLS_GX
    units = []
    kc = 0
    si = 0
    for bi in range(10):         # (BIG, small) x 10 -> 3 kc each
        units.append(('BIGG' if bi in BIGG_IDX else 'BIG', kc)); kc += 2
        units.append((smalls[si], kc)); kc += 1
        si += 1
    units.append((smalls[si], kc)); kc += 1; si += 1
    units.append((smalls[si], kc)); kc += 1; si += 1
    assert kc == NKC and si == 12
    return units


def _split_multi_waits(nc):
    """Walrus encodes at most one sync-wait per instruction; move extras onto
    single-wait NoOps inserted before the instruction on the same engine."""
    ctr = 0
    for f in nc.m.functions:
        for bb in f.blocks:
            il = bb.instructions
            i = 0
            while i < len(il):
                ins = il[i]
                si = ins.sync_info
                if si is not None and len(si.on_wait) > 1:
                    waits = list(si.on_wait)
                    ins.sync_info = mybir.SyncInfo(
                        on_wait=[waits[-1]], on_update=list(si.on_update)
                    )
                    for w in waits[:-1]:
                        ctr += 1
                        nop = mybir.InstNoOp(
                            name=f"I-waitsplit-{ctr}", ins=[], outs=[]
                        )
                        nop.engine = ins.engine
                        nop.sync_info = mybir.SyncInfo(on_wait=[w], on_update=[])
                        il.insert(i, nop)
                        i += 1
                i += 1


def build_program(split: bool = True, debug: bool = False) -> bass.Bass:
    nc = bass.Bass()

    xT_in = nc.dram_tensor("xT_in", [D, N], bf16, kind="ExternalInput")
    xrT_in = nc.dram_tensor("xrT_in", [D, QR], bf16, kind="ExternalInput")
    wqT_in = nc.dram_tensor("wqT_in", [D, D], bf16, kind="ExternalInput")
    wkT_in = nc.dram_tensor("wkT_in", [D, D], bf16, kind="ExternalInput")
    wvT_in = nc.dram_tensor("wvT_in", [D, D], bf16, kind="ExternalInput")
    # [x rows (256) | per-group y row blocks (4 x 128, zero-padded)]
    wpT_in = nc.dram_tensor("wpT_in", [6 * 128, D], bf16, kind="ExternalInput")
    bq_in = nc.dram_tensor("bq_in", [1, D], bf16, kind="ExternalInput")
    bk_in = nc.dram_tensor("bk_in", [1, D], bf16, kind="ExternalInput")
    bp_in = nc.dram_tensor("bp_in", [1, D], bf16, kind="ExternalInput")
    mask_in = nc.dram_tensor("mask_in", [128, NKC * QR], bf16, kind="ExternalInput")
    mask8_in = nc.dram_tensor("mask8_in", [128, NKC * QR], f8, kind="ExternalInput")
    out = nc.dram_tensor("out", [QR, D], f32, kind="ExternalOutput")
    if debug:
        dbg = {
            nm: nc.dram_tensor(f"dbg_{nm}", shp, dt, kind="ExternalOutput")
            for nm, shp, dt in [
                ("qt0", [128, QR], bf16),
                ("kt0", [128, QR], bf16),
                ("vaug", [128, 2 * 272], bf16),
                ("mask", [128, 1024], bf16),
                ("praw", [128, 2048], bf16),
                ("phat", [128, 2048], bf16),
                ("u", [128, 1024], bf16),
                ("p2", [128, 1024], bf16),
                ("yst0", [128, QR], f32),
                ("rzb0", [66, QR], bf16),
                ("yg0", [128, QR], bf16),
            ]
        }

    with tile.TileContext(nc) as tc:
        with (
            tc.tile_pool(name="cons", bufs=1) as cons,
            tc.tile_pool(name="big", bufs=1) as big,
            tc.tile_pool(name="work", bufs=3) as work,
        ):
            nc.gpsimd.load_library(library_config.standard)

            xrT = [cons.tile([128, QR], bf16, tag=f"xrT{i}", name=f"xrT{i}") for i in range(2)]
            wqT = [cons.tile([128, D], bf16, tag=f"wqT{i}", name=f"wqT{i}") for i in range(2)]
            xT = [cons.tile([128, N], bf16, tag=f"xT{i}", name=f"xT{i}") for i in range(2)]
            wkT = [cons.tile([128, D], bf16, tag=f"wkT{i}", name=f"wkT{i}") for i in range(2)]
            wvT = [cons.tile([128, D], bf16, tag=f"wvT{i}", name=f"wvT{i}") for i in range(2)]
            wpT = [cons.tile([128, D], bf16, tag=f"wpT{i}", name=f"wpT{i}") for i in range(6)]
            # DMAs spread across four engine queues so the preamble is not
            # serialized on one ring: sync carries x/xr/wq/wk (PE-gating),
            # scalar+gpsimd carry mask halves, vector carries wv/wp/biases.
            for i in range(2):
                nc.sync.dma_start(out=xrT[i][:], in_=xrT_in[i * 128 : (i + 1) * 128, :])
                nc.sync.dma_start(out=wqT[i][:], in_=wqT_in[i * 128 : (i + 1) * 128, :])
            for i in range(2):
                nc.sync.dma_start(out=wkT[i][:], in_=wkT_in[i * 128 : (i + 1) * 128, :])
            for i in range(2):
                for h in range(4):
                    nc.sync.dma_start(
                        out=xT[i][:, h * 1024 : (h + 1) * 1024],
                        in_=xT_in[i * 128 : (i + 1) * 128, h * 1024 : (h + 1) * 1024],
                    )
            for i in range(2):
                nc.gpsimd.dma_start(out=wvT[i][:], in_=wvT_in[i * 128 : (i + 1) * 128, :])
            # dense 0/1 mask^T in 4-chunk pieces, alternating two queues so
            # chunk k lands well before unit k consumes it
            mask_sb = big.tile([128, NKC * QR], bf16)
            for mb_ in range(8):
                eng = nc.scalar if mb_ % 2 == 0 else nc.gpsimd
                eng.dma_start(
                    out=mask_sb[:, mb_ * 4 * QR : (mb_ + 1) * 4 * QR],
                    in_=mask_in[:, mb_ * 4 * QR : (mb_ + 1) * 4 * QR],
                )
            mask8_sb = big.tile([128, NKC * QR], f8)
            for mb_ in range(4):
                nc.gpsimd.dma_start(
                    out=mask8_sb[:, mb_ * 8 * QR : (mb_ + 1) * 8 * QR],
                    in_=mask8_in[:, mb_ * 8 * QR : (mb_ + 1) * 8 * QR],
                )
            for i in range(6):
                nc.scalar.dma_start(out=wpT[i][:], in_=wpT_in[i * 128 : (i + 1) * 128, :])
            bq_sb = cons.tile([1, D], bf16)
            bk_sb = cons.tile([1, D], bf16)
            bp_sb = cons.tile([1, D], bf16)
            nc.scalar.dma_start(out=bq_sb[:], in_=bq_in[:])
            nc.scalar.dma_start(out=bk_sb[:], in_=bk_in[:])
            nc.scalar.dma_start(out=bp_sb[:], in_=bp_in[:])

            ones_bf = cons.tile([1, QR], bf16)
            nc.vector.memset(ones_bf[:], 1.0)
            ones128 = cons.tile([128, 128], bf16)
            nc.vector.memset(ones128[:], 1.0)
            # sel: 1/Z broadcast.  Z rows 32 (head0) / 96 (head1) fan out to
            # y rows 0-31 / 64-95; all other output rows get zeros.
            sel_sb = cons.tile([128, 128], bf16)
            nc.vector.memset(sel_sb[:], 0.0)
            nc.vector.memset(sel_sb[32:33, 0:32], 1.0)
            nc.vector.memset(sel_sb[96:97, 64:96], 1.0)

            QT = [big.tile([128, QR], bf16, tag=f"QT{i}", name=f"QT{i}") for i in range(2)]
            KT = [big.tile([128, N], bf16, tag=f"KT{i}", name=f"KT{i}") for i in range(2)]
            # V blocks: [128 nodes, kc*8*VB + h*VB + (32 V cols | ones | pad)]
            Vaug = big.tile([128, NKC * H * VB], f8)
            vv = Vaug[:].rearrange("p (kc h c) -> p kc h c", kc=NKC, h=H)
            nc.vector.memset(vv[:, :, :, 32:33], 1.0)

            ystage = [
                cons.tile([128, QR], f32, tag=f"yst{i}", name=f"yst{i}")
                for i in range(4)
            ]
            yg = [cons.tile([128, QR], bf16, tag=f"yg{i}", name=f"yg{i}") for i in range(4)]
            rz = cons.tile([128, QR], f32, tag="rz", name="rz")
            rzb = cons.tile([128, QR], bf16, tag="rzb", name="rzb")

            def mask3(kc):
                return (
                    mask_sb[:, kc * QR : (kc + 1) * QR]
                    .unsqueeze(1)
                    .broadcast_to([128, 2, QR])
                )

            def mask4(kc0):
                # [p, 2(head rep, stride 0), 2(kc), q]
                return (
                    mask_sb[:, kc0 * QR : (kc0 + 2) * QR]
                    .rearrange("p (kc q) -> p kc q", kc=2)
                    .unsqueeze(1)
                    .broadcast_to([128, 2, 2, QR])
                )

            with tc.tile_pool(name="psa", bufs=1, space="PSUM") as psa:
                pvt = [None]

                def st_mm(sp_slice, g, kc, h):
                    band = (h % 4) * DH
                    dt_ = g // 2
                    nc.tensor.matmul(
                        sp_slice,
                        lhsT=KT[dt_][band : band + DH, kc * 128 : (kc + 1) * 128],
                        rhs=QT[dt_][band : band + DH, :],
                        start=True,
                        stop=True,
                        tile_position=(band, 0),
                    )

                def pv_mm(g, kc, rhs_pair, first, last, rhs_is_mask=False):
                    """Two PV accumulants: head j of group g contributes its
                    32 y rows + Z row via a 33-col [V|ones] block."""
                    pv = pvt[0]
                    for j in range(2):
                        h = 2 * g + j
                        base = (kc * H + h) * VB
                        rhs = rhs_pair if rhs_is_mask else rhs_pair[:, j * QR : (j + 1) * QR]
                        nc.tensor.matmul(
                            pv[64 * j : 64 * j + 33, :],
                            lhsT=Vaug[:, base : base + 33],
                            rhs=rhs,
                            start=first,
                            stop=last,
                            tile_position=(0, 64 * j),
                        )

                def emit_big(g, kc0, last, on_gp=False):
                    # head-major layout [h0 kc0 | h0 kc1 | h1 kc0 | h1 kc1]
                    # so each head's 2-chunk pair is one stride-1024 AP for
                    # the fp8 DoubleRow PV matmul.
                    h0, h1 = 2 * g, 2 * g + 1
                    sp = psa.tile([128, 2048], f32, tag="spb", bufs=1)
                    for j, (kc, h) in enumerate(
                        ((kc0, h0), (kc0 + 1, h0), (kc0, h1), (kc0 + 1, h1))
                    ):
                        st_mm(sp[:, j * QR : (j + 1) * QR], g, kc, h)
                    praw = work.tile([128, 2048], bf16, tag="prb", bufs=2)
                    nc.scalar.activation(praw[:], sp[:], AF.Exp, scale=SCALE)
                    phat = work.tile([128, 2048], f8, tag="phb", bufs=2)
                    if on_gp:
                        for j, kc in enumerate((kc0, kc0 + 1, kc0, kc0 + 1)):
                            nc.gpsimd.tensor_tensor(
                                out=phat[:, j * QR : (j + 1) * QR],
                                in0=praw[:, j * QR : (j + 1) * QR],
                                in1=mask_sb[:, kc * QR : (kc + 1) * QR],
                                op=OP.mult,
                            )
                    else:
                        nc.vector.tensor_tensor(
                            out=phat[:].rearrange("p (b a q) -> p b a q", b=2, a=2),
                            in0=praw[:].rearrange("p (b a q) -> p b a q", b=2, a=2),
                            in1=mask4(kc0),
                            op=OP.mult,
                        )
                    if debug and g == 0 and kc0 == 0:
                        nc.sync.dma_start(out=dbg["praw"][:], in_=praw[:])
                        nc.sync.dma_start(out=dbg["phat"][:], in_=phat[:])
                    pv = pvt[0]
                    vv4 = Vaug[:].rearrange("p (kc h c) -> p kc h c", kc=NKC, h=H)
                    for j in range(2):
                        h = 2 * g + j
                        nc.tensor.matmul(
                            pv[64 * j : 64 * j + 33, :],
                            lhsT=vv4[:, kc0 : kc0 + 2, h, 0:33],
                            rhs=phat[:, j * 1024 : (j + 1) * 1024].rearrange(
                                "p (a q) -> p a q", a=2
                            ),
                            start=(kc0 == 0),
                            stop=last,
                            tile_position=(0, 64 * j),
                            perf_mode=mybir.MatmulPerfMode.DoubleRow,
                        )

                def emit_small(g, kc, kind, last):
                    h0, h1 = 2 * g, 2 * g + 1
                    sp = psa.tile([128, 1024], f32, tag="sps", bufs=1)
                    st_mm(sp[:, 0:QR], g, kc, h0)
                    st_mm(sp[:, QR : 2 * QR], g, kc, h1)
                    if kind == 'B':
                        U = work.tile([128, 1024], bf16, tag="ub", bufs=2)
                        nc.vector.scalar_tensor_tensor(
                            out=U[:].rearrange("p (a q) -> p a q", a=2),
                            in0=sp[:].rearrange("p (a q) -> p a q", a=2),
                            scalar=A2,
                            in1=mask3(kc),
                            op0=OP.mult,
                            op1=OP.mult,
                        )
                        P2 = work.tile([128, 1024], f8, tag="p2b", bufs=2)
                        nc.vector.scalar_tensor_tensor(
                            out=P2[:],
                            in0=U[:],
                            scalar=2.0,
                            in1=U[:],
                            op0=OP.add,
                            op1=OP.mult,
                        )
                        if debug and g == 0 and kc == 14:
                            nc.sync.dma_start(out=dbg["u"][:], in_=U[:])
                            nc.sync.dma_start(out=dbg["p2"][:], in_=P2[:])
                        pv_mm(g, kc, P2[:], kc == 0, False)
                        # mask term: exp(s) ~ 1 + 2U + U^2, the "1" rides the
                        # PE with the head-shared fp8 mask as rhs
                        pv_mm(
                            g, kc, mask8_sb[:, kc * QR : (kc + 1) * QR],
                            False, last, rhs_is_mask=True,
                        )
                    else:
                        praw = work.tile([128, 1024], bf16, tag="prs", bufs=2)
                        nc.scalar.activation(praw[:], sp[:], AF.Exp, scale=SCALE)
                        phat = work.tile([128, 1024], f8, tag="phs", bufs=2)
                        if kind == 'G':
                            mkc = mask_sb[:, kc * QR : (kc + 1) * QR]
                            for c0 in (0, QR):
                                nc.gpsimd.tensor_tensor(
                                    out=phat[:, c0 : c0 + QR],
                                    in0=praw[:, c0 : c0 + QR],
                                    in1=mkc,
                                    op=OP.mult,
                                )
                        else:
                            nc.vector.tensor_tensor(
                                out=phat[:].rearrange("p (a q) -> p a q", a=2),
                                in0=praw[:].rearrange("p (a q) -> p a q", a=2),
                                in1=mask3(kc),
                                op=OP.mult,
                            )
                        pv_mm(g, kc, phat[:], kc == 0, last)

                def emit_group(g):
                    for kind, kc in _unit_list(g):
                        if kind in ('BIG', 'BIGG'):
                            emit_big(g, kc, kc + 1 == NKC - 1, kind == 'BIGG')
                        else:
                            emit_small(g, kc, kind, kc == NKC - 1)

                def drain_group(g, rp):
                    pv = pvt[0]
                    nc.vector.tensor_copy(ystage[g][:], pv[:])
                    # K=98 contraction from partition 0 (row-offset
                    # tile_position requires K<=64); rows with zero sel
                    # weights contribute nothing, and recip of the finite
                    # non-Z rows stays finite.
                    nc.vector.reciprocal_approx_fast(rz[0:98, :], pv[0:98, :])
                    nc.vector.tensor_copy(rzb[0:98, :], rz[0:98, :])
                    nc.tensor.matmul(
                        rp[:],
                        lhsT=sel_sb[0:98, :],
                        rhs=rzb[0:98, :],
                        start=True,
                        stop=True,
                        tile_position=(0, 0),
                    )
                    if debug and g == 0:
                        nc.sync.dma_start(out=dbg["yst0"][:], in_=ystage[0][:])
                        nc.sync.dma_start(out=dbg["rzb0"][:], in_=rzb[32:98, :])
                    nc.vector.tensor_tensor(
                        out=yg[g][:], in0=ystage[g][:], in1=rp[:], op=OP.mult
                    )
                    if debug and g == 0:
                        nc.sync.dma_start(out=dbg["yg0"][:], in_=yg[0][:])

                with tc.tile_pool(name="psp", bufs=1, space="PSUM") as psp:
                    # PE warm-up doubles as pv pre-fill: every partition of pv
                    # gets a finite value before the Z-recip reads untouched
                    # rows.
                    pvt[0] = psa.tile([128, QR], f32, tag="pv", bufs=1, name="pv_g0")
                    for r in range(12):
                        j = r % 4
                        nc.tensor.matmul(
                            pvt[0][:, j * 128 : (j + 1) * 128],
                            lhsT=ones128[:],
                            rhs=ones128[:],
                            start=True,
                            stop=True,
                        )

                    def bias_mm(ps, brow, dt_):
                        nc.tensor.matmul(
                            ps,
                            lhsT=brow[0:1, dt_ * 128 : (dt_ + 1) * 128],
                            rhs=ones_bf[0:1, :],
                            start=False,
                            stop=True,
                            tile_position=(0, 0),
                        )

                    # Q projection
                    for dt_ in range(2):
                        qp = psp.tile([128, QR], f32, tag="proj", bufs=1)
                        for cc in range(2):
                            nc.tensor.matmul(
                                qp[:],
                                lhsT=wqT[cc][:, dt_ * 128 : (dt_ + 1) * 128],
                                rhs=xrT[cc][:],
                                start=(cc == 0),
                                stop=False,
                            )
                        bias_mm(qp[:], bq_sb, dt_)
                        nc.vector.tensor_copy(QT[dt_][:], qp[:])
                        if debug and dt_ == 0:
                            nc.sync.dma_start(out=dbg["qt0"][:], in_=QT[0][:])

                    # K/V projections interleaved with group-0 attention
                    units0 = _unit_list(0)
                    ui = 0
                    for nch in range(8):
                        for dt_ in range(2):
                            kp = psp.tile([128, QR], f32, tag="proj", bufs=1)
                            for cc in range(2):
                                nc.tensor.matmul(
                                    kp[:],
                                    lhsT=wkT[cc][:, dt_ * 128 : (dt_ + 1) * 128],
                                    rhs=xT[cc][:, nch * QR : (nch + 1) * QR],
                                    start=(cc == 0),
                                    stop=False,
                                )
                            bias_mm(kp[:], bk_sb, dt_)
                            nc.vector.tensor_copy(
                                KT[dt_][:, nch * QR : (nch + 1) * QR], kp[:]
                            )
                            if debug and nch == 0 and dt_ == 0:
                                nc.sync.dma_start(
                                    out=dbg["kt0"][:], in_=KT[0][:, 0:QR]
                                )
                        for nb in range(nch * 4, nch * 4 + 4):
                            vp = psp.tile([128, QR], f32, tag="proj", bufs=1)
                            for cc in range(2):
                                nc.tensor.matmul(
                                    vp[:, 0:D],
                                    lhsT=xT[cc][:, nb * 128 : (nb + 1) * 128],
                                    rhs=wvT[cc][:],
                                    start=(cc == 0),
                                    stop=(cc == 1),
                                )
                            veng = nc.vector if nb % 2 == 0 else nc.scalar
                            if nb % 2 == 0:
                                nc.vector.tensor_copy(
                                    Vaug[:, nb * H * VB : (nb + 1) * H * VB]
                                    .rearrange("p (h c) -> p h c", h=H)[:, :, 0:32],
                                    vp[:, 0:D].rearrange("p (h c) -> p h c", h=H),
                                )
                            else:
                                nc.scalar.copy(
                                    Vaug[:, nb * H * VB : (nb + 1) * H * VB]
                                    .rearrange("p (h c) -> p h c", h=H)[:, :, 0:32],
                                    vp[:, 0:D].rearrange("p (h c) -> p h c", h=H),
                                )
                            if debug and nb == 1:
                                nc.sync.dma_start(
                                    out=dbg["vaug"][:], in_=Vaug[:, 0 : 2 * H * VB]
                                )
                                nc.sync.dma_start(
                                    out=dbg["mask"][:], in_=mask_sb[:, 0:1024]
                                )
                        kc_ready = nch * 4 + 4
                        while ui < len(units0):
                            kind, kc = units0[ui]
                            span = 2 if kind in ('BIG', 'BIGG') else 1
                            if kc + span > kc_ready:
                                break
                            if kind in ('BIG', 'BIGG'):
                                emit_big(0, kc, kc + 1 == NKC - 1, kind == 'BIGG')
                            else:
                                emit_small(0, kc, kind, kc == NKC - 1)
                            ui += 1
                    assert ui == len(units0)

                with tc.tile_pool(name="psr", bufs=1, space="PSUM") as psr:
                    rp = psr.tile([128, QR], f32, tag="rp", bufs=1, name="rp")
                    drain_group(0, rp)
                    for g in range(1, 4):
                        pvt[0] = psa.tile(
                            [128, QR], f32, tag="pv", bufs=1, name=f"pv_g{g}"
                        )
                        emit_group(g)
                        drain_group(g, rp)

            # ---------- final projection ----------
            with tc.tile_pool(name="pso", bufs=1, space="PSUM") as pso:
                catT = [xrT[0], xrT[1], yg[0], yg[1], yg[2], yg[3]]
                for qb in range(4):
                    op_ = pso.tile([128, D], f32, tag="op", bufs=2)
                    for cc in range(6):
                        nc.tensor.matmul(
                            op_[:],
                            lhsT=catT[cc][:, qb * 128 : (qb + 1) * 128],
                            rhs=wpT[cc][:],
                            start=(cc == 0),
                            stop=False,
                        )
                    nc.tensor.matmul(
                        op_[:],
                        lhsT=ones_bf[:, 0:128],
                        rhs=bp_sb[:],
                        start=False,
                        stop=True,
                    )
                    osb = work.tile([128, D], f32, tag="osb")
                    nc.vector.tensor_copy(osb[:], op_[:])
                    nc.sync.dma_start(
                        out=out[qb * 128 : (qb + 1) * 128, :], in_=osb[:]
                    )

    lower_extended_insts(nc)
    if split:
        _split_multi_waits(nc)
    return nc


_PROGRAM = None


def _get_program():
    global _PROGRAM
    if _PROGRAM is None:
        _PROGRAM = build_program()
    return _PROGRAM


def shard_inputs(inputs):
    bf = ml_dtypes.bfloat16
    x = np.asarray(inputs["x"], np.float32)
    ei = np.asarray(inputs["edge_index"])
    src = ei[0].astype(np.int64)   # query node of each edge
    dst = ei[1].astype(np.int64)   # key node of each edge
    Wq = np.asarray(inputs["Wq"], np.float32)
    Wk = np.asarray(inputs["Wk"], np.float32)
    Wv = np.asarray(inputs["Wv"], np.float32)
    Wp = np.asarray(inputs["Wp"], np.float32)
    bq = np.asarray(inputs["bq"], np.float32)
    bk = np.asarray(inputs["bk"], np.float32)
    bv = np.asarray(inputs["bv"], np.float32)
    bp = np.asarray(inputs["bp"], np.float32)

    xT = np.ascontiguousarray(x.T.astype(bf))                      # [256, 4096]
    wqT = np.ascontiguousarray(Wq.T.astype(bf))                    # [256, 256]
    wkT = np.ascontiguousarray(Wk.T.astype(bf))
    wvT = np.ascontiguousarray(Wv.T.astype(bf))
    bq1 = np.ascontiguousarray(bq.reshape(1, D).astype(bf))
    bk1 = np.ascontiguousarray(bk.reshape(1, D).astype(bf))
    # softmax rows sum to 1: fold the V bias into the output bias.
    bp_f = bp + bv @ Wp[:, D : 2 * D].T
    bp1 = np.ascontiguousarray(bp_f.reshape(1, D).astype(bf))

    # wpT blocks: x rows then per-group y rows at their pv positions
    # (yg_g row r = head 2g dim r; row 64+r = head 2g+1 dim r; rest zero)
    WpT = Wp.T                                                      # [512, 256]
    wp6 = np.zeros((6 * 128, D), np.float32)
    wp6[0:256] = WpT[0:256]
    for g in range(4):
        blk = wp6[(2 + g) * 128 : (3 + g) * 128]
        blk[0:32] = WpT[D + 64 * g : D + 64 * g + 32]
        blk[64:96] = WpT[D + 64 * g + 32 : D + 64 * g + 64]
    wp6 = np.ascontiguousarray(wp6.astype(bf))

    # dense 0/1 mask^T: mask[k_local, kc*QR + q] = edge(query q0+q -> key)
    allowed = np.zeros((N, N), np.bool_)                           # [key, query]
    allowed[dst, src] = True

    in_maps = []
    for c in range(NCORES):
        q0 = c * QR
        mc = allowed[:, q0 : q0 + QR]                              # [4096, 512]
        mh = np.ascontiguousarray(
            mc.reshape(NKC, 128, QR).transpose(1, 0, 2).reshape(128, NKC * QR)
            .astype(bf)
        )
        in_maps.append(
            {
                "xT_in": xT,
                "xrT_in": np.ascontiguousarray(xT[:, q0 : q0 + QR]),
                "wqT_in": wqT, "wkT_in": wkT, "wvT_in": wvT, "wpT_in": wp6,
                "bq_in": bq1, "bk_in": bk1, "bp_in": bp1,
                "mask_in": mh,
                "mask8_in": np.ascontiguousarray(
                    mh.astype(ml_dtypes.float8_e4m3fn)
                ),
            }
        )
    return in_maps


def run(inputs, trace=False):
    from concourse.bass_utils import run_bass_kernel_spmd

    nc = _get_program()
    in_maps = shard_inputs(inputs)
    res = run_bass_kernel_spmd(nc, in_maps, core_ids=list(range(NCORES)), trace=trace)
    full = np.concatenate([res.results[c]["out"] for c in range(NCORES)], axis=0)
    return np.ascontiguousarray(full.astype(np.float32)), res


def kernel(**inputs) -> np.ndarray:
    out, _ = run(inputs, trace=False)
    return out
